# revision 32
# baseline (speedup 1.0000x reference)
"""Trainium2 Bass kernel for nn_Decoder (attention-LSTM decoder).

Data-parallel over batch B=128 across 8 NeuronCores (16 rows each).
All LayerNorm gains/biases and additive biases are folded host-side:
LN gamma/beta fold into downstream weight matrices; additive biases ride
as augmented contraction rows (the activation side carries a ones row).
Device states are the standardized (pre-gamma/beta) h values.
"""
import sys
sys.path.insert(0, "/opt/trn_rl_repo")

import numpy as np

import jax
from jax.sharding import Mesh, PartitionSpec, NamedSharding
from jax.experimental.shard_map import shard_map

import concourse.bass as bass
import concourse.bacc as bacc
import concourse.tile as tile
import concourse.mybir as mybir
from concourse.bass2jax import _bass_exec_p, install_neuronx_cc_hook, partition_id_tensor
from concourse.masks import make_identity

F32 = mybir.dt.float32
F16 = mybir.dt.float16
AF = mybir.ActivationFunctionType
ALU = mybir.AluOpType

NCORES = 8
B, T, N = 128, 20, 49          # full batch, seq len, attention positions
TS = T - 1                      # 19 decode steps
BL = B // NCORES                # 16 local batch
VOCAB, EMB, H, VDIM, ATT = 10000, 512, 1024, 512, 512
G = 4 * H                       # 4096 gate dim
LN_EPS = 1e-5
VP = 20 * 512                   # padded vocab 10240
ROWS = TS * BL                  # 304 output rows per core (t-major)
BN = BL * N                     # 784 local (b, n) rows
BNT = 98                        # bn tile rows (2 batches of 49)
NBNT = BN // BNT                # 8 bn tiles
f16 = np.float16

_cache = {}
DEBUG = False


def _f16(x):
    return np.ascontiguousarray(np.asarray(x, np.float32).astype(f16))


def _f32(x):
    return np.ascontiguousarray(np.asarray(x, np.float32))


def _ktile(w, p=128):
    """[K, M] row-major -> [p, K//p, M] SBUF k-tile image (partition-major)."""
    k, m = w.shape
    return np.ascontiguousarray(w.reshape(k // p, p, m).transpose(1, 0, 2))


def _aug(w, biasrow):
    """Append bias as contraction row 0 of one extra 128-row k-tile."""
    k, m = w.shape
    out = np.zeros((k + 128, m), w.dtype)
    out[:k] = w
    out[k] = biasrow
    return out


def _nmaj(w):
    """[K, M] -> [M//512, 128, K//128, 512] (n-major stream layout)."""
    k, m = w.shape
    return np.ascontiguousarray(
        w.reshape(k // 128, 128, m // 512, 512).transpose(2, 1, 0, 3))


def build_program():
    nc = bacc.Bacc("TRN2", target_bir_lowering=False, debug=False,
                   num_devices=NCORES)
    D = {}

    def din(name, shape, dt):
        D[name] = nc.dram_tensor(name, list(shape), dt, kind="ExternalInput").ap()
        return D[name]

    # per-core activations (augmented with ones row in k-tile 4)
    din("VT", [128, 5, BN], F16)
    din("xT", [128, 5, ROWS], F16)
    # shared params
    din("VpWT", [128, 5, H], F16)         # aug: Vp_b
    din("M1", [128, 4, ATT], F16)         # Vp_W.T @ attU.T
    din("bU", [128, 4], F32)
    din("Amean", [BNT, NBNT, BL], F16)
    din("ihWT", [2, 128, 9, 512], F16)    # aug: ih_b; n-major
    din("icWT", [2, 128, 9, 512], F16)    # aug: ic_b; n-major
    din("invG1", [BL, H], F32)
    din("nB1G1", [BL, H], F32)
    din("W1embT", [8, 128, 5, 512], F16)  # aug: bias1; n-major
    din("W1ctxT", [8, 128, 8, 512], F16)  # streamed per step; n-major
    din("Whh1T", [8, 128, 8, 512], F16)   # streamed per step; n-major
    din("W2ihT", [8, 128, 9, 512], F16)   # streamed; aug bias2; n-major
    din("Whh2T", [128, 8, G], F16)        # resident
    din("attWT", [128, 9, ATT], F16)      # resident; aug: bq
    din("attv", [128, 4], F16)
    din("projWT", [128, 9, VDIM], F16)    # resident; aug: bp
    din("ET", [20, 128, 4, 512], F16)     # embed_W.T padded
    din("h2Tinit", [128, 9, BL], F16)     # standardized h2 init + ones tile
    out_ap = nc.dram_tensor("out", [ROWS, VP], F32, kind="ExternalOutput").ap()
    dbg = {}
    if DEBUG:
        dbg["vp"] = nc.dram_tensor("dbg_vp", [BNT, NBNT, H], F16,
                                   kind="ExternalOutput").ap()
        dbg["uvT"] = nc.dram_tensor("dbg_uvT", [128, 4, BN], F16,
                                    kind="ExternalOutput").ap()
        dbg["fm"] = nc.dram_tensor("dbg_fm", [BL, H], F32,
                                   kind="ExternalOutput").ap()
        dbg["c1"] = nc.dram_tensor("dbg_c1", [BL, H], F32,
                                   kind="ExternalOutput").ap()
        dbg["h1T"] = nc.dram_tensor("dbg_h1T", [128, 9, BL], F16,
                                    kind="ExternalOutput").ap()
        dbg["xw"] = nc.dram_tensor("dbg_xw", [BL, G], F16,
                                   kind="ExternalOutput").ap()
        dbg["q0"] = nc.dram_tensor("dbg_q0", [BL, ATT], F32,
                                   kind="ExternalOutput").ap()
        dbg["ex0"] = nc.dram_tensor("dbg_ex0", [1, BN], F32,
                                    kind="ExternalOutput").ap()
        dbg["ctx0"] = nc.dram_tensor("dbg_ctx0", [BL, H], F32,
                                     kind="ExternalOutput").ap()
        dbg["g10"] = nc.dram_tensor("dbg_g10", [BL, G], F32,
                                    kind="ExternalOutput").ap()
        dbg["s10"] = nc.dram_tensor("dbg_s10", [128, 9, BL], F16,
                                    kind="ExternalOutput").ap()
        dbg["s20"] = nc.dram_tensor("dbg_s20", [128, 9, BL], F16,
                                    kind="ExternalOutput").ap()

    with tile.TileContext(nc) as tc:
        _emit(nc, tc, D, out_ap, dbg)
    nc.compile()
    return nc


def _emit(nc, tc, D, out_ap, dbg=None):
    import contextlib
    ctx = contextlib.ExitStack()
    with ctx:
        consts = ctx.enter_context(tc.tile_pool(name="consts", bufs=1))
        resid = ctx.enter_context(tc.tile_pool(name="resid", bufs=1))
        stream = ctx.enter_context(tc.tile_pool(name="stream", bufs=2))
        xwp = ctx.enter_context(tc.tile_pool(name="xwp", bufs=1))
        states = ctx.enter_context(tc.tile_pool(name="states", bufs=1))
        trans = ctx.enter_context(tc.tile_pool(name="trans", bufs=1))
        hT = ctx.enter_context(tc.tile_pool(name="hT", bufs=2))
        outp = ctx.enter_context(tc.tile_pool(name="outp", bufs=2))
        psA = ctx.enter_context(tc.tile_pool(name="psA", bufs=2, space="PSUM"))
        psB = ctx.enter_context(tc.tile_pool(name="psB", bufs=4, space="PSUM"))
        dramp = ctx.enter_context(tc.tile_pool(name="dramp", bufs=1, space="DRAM"))

        sync, ve, se, te = nc.sync, nc.vector, nc.scalar, nc.tensor

        # ---- constants ----
        ident = consts.tile([128, 128], F32)
        make_identity(nc, ident)
        i16 = consts.tile([16, 16], F16)
        make_identity(nc, i16)
        eps = consts.tile([BL, 1], F32)
        ve.memset(eps, LN_EPS)
        attv_t = consts.tile([128, 4], F16)
        sync.dma_start(out=attv_t, in_=D["attv"])

        uvT = resid.tile([128, 4, BN], F16)
        vp = resid.tile([BNT, NBNT, H], F16)
        aM = resid.tile([BNT, NBNT, BL], F16)
        sync.dma_start(out=aM, in_=D["Amean"])
        amat = resid.tile([BNT, NBNT, BL], F16)
        ve.memset(amat, 0.0)
        ptT = resid.tile([128, 4, ROWS], F16)
        c1 = states.tile([BL, H], F32)
        c2 = states.tile([BL, H], F32)
        ve.memset(c2, 0.0)
        xw_dram = dramp.tile([3, 128, G], F16)

        def ps_copy(dst, src, engine=ve):
            engine.tensor_copy(out=dst, in_=src)

        def transpose_16cols(src_sb, n128, ps_tile):
            """src_sb [16, n128*128] f32 -> ps_tile [128, n128*16] f32."""
            for k in range(n128):
                te.transpose(ps_tile[:, k * 16:(k + 1) * 16],
                             src_sb[:, k * 128:(k + 1) * 128], ident[:16, :16])

        def fresh_hT(tag):
            """New h-state tile [128, 9, 16] f16 with ones row in tile 8."""
            t_ = hT.tile([128, 9, BL], F16, tag=tag)
            ve.memset(t_[:, 8, :], 0.0)
            ve.memset(t_[0:1, 8, :], 1.0)
            return t_

        # =============== phase 0 ===============
        with tc.tile_pool(name="ph0", bufs=1) as ph0:
            vtT = ph0.tile([128, 5, BN], F16)
            sync.dma_start(out=vtT, in_=D["VT"])
            xtT = ph0.tile([128, 5, ROWS], F16)
            sync.dma_start(out=xtT, in_=D["xT"])
            vpwT = ph0.tile([128, 5, H], F16)
            sync.dma_start(out=vpwT, in_=D["VpWT"])
            m1 = ph0.tile([128, 4, ATT], F16)
            sync.dma_start(out=m1, in_=D["M1"])
            bu = ph0.tile([128, 4], F32)
            sync.dma_start(out=bu, in_=D["bU"])

            # Vp = V @ Vp_W.T + Vp_b   -> vp [98, 8, 1024] f16
            for m in range(NBNT):
                for n in range(2):
                    ps = psB.tile([BNT, 512], F32, tag="b1")
                    for k in range(5):
                        te.matmul(ps, vtT[:, k, m * BNT:(m + 1) * BNT],
                                  vpwT[:, k, n * 512:(n + 1) * 512],
                                  start=(k == 0), stop=(k == 4))
                    ps_copy(vp[:, m, n * 512:(n + 1) * 512], ps)

            # UvT = M1.T @ VT + bU -> uvT [128, 4, 784] f16
            for j in range(4):
                for hh in range(2):
                    ps = psB.tile([128, 512], F32, tag="b1")
                    for k in range(4):
                        te.matmul(ps[:, :392],
                                  m1[:, k, j * 128:(j + 1) * 128],
                                  vtT[:, k, hh * 392:(hh + 1) * 392],
                                  start=(k == 0), stop=(k == 3))
                    ve.tensor_scalar(
                        out=uvT[:, j, hh * 392:(hh + 1) * 392],
                        in0=ps[:, :392], scalar1=bu[:, j:j + 1],
                        scalar2=None, op0=ALU.add)

            # feat_mean = Amean.T @ Vp  [16, 1024]
            fm = trans.tile([BL, H], F32, tag="cellg")
            for n in range(2):
                psf = psB.tile([BL, 512], F32, tag="b1")
                for i in range(NBNT):
                    te.matmul(psf, aM[:, i, :],
                              vp[:, i, n * 512:(n + 1) * 512],
                              start=(i == 0), stop=(i == NBNT - 1))
                ps_copy(fm[:, n * 512:(n + 1) * 512], psf)
            fmT_ps = psB.tile([128, 8 * 16], F32, tag="b1")
            transpose_16cols(fm, 8, fmT_ps)
            fmT = hT.tile([128, 9, BL], F16, tag="fmT")
            ps_copy(fmT[:, :8, :], fmT_ps.rearrange("p (k b) -> p k b", b=16))
            ve.memset(fmT[:, 8, :], 0.0)
            ve.memset(fmT[0:1, 8, :], 1.0)

            # h1 = tanh(fm @ ihW.T + ihb); s1 = (h1 - b1)/g1
            # c1 = tanh(fm @ icW.T + icb)
            ig1nb1 = trans.tile([BL, 2, H], F32, tag="cellh")
            ig1, nb1 = ig1nb1[:, 0, :], ig1nb1[:, 1, :]
            sync.dma_start(out=ig1, in_=D["invG1"])
            sync.dma_start(out=nb1, in_=D["nB1G1"])
            s1r = trans.tile([BL, H], F32, tag="cellg")
            h1T = None
            for which, wname in (("h", "ihWT"), ("c", "icWT")):
                for n in range(2):
                    wst = stream.tile([128, 9, 512], F16, tag="wst")
                    sync.dma_start(out=wst, in_=D[wname][n])
                    ps = psB.tile([BL, 512], F32, tag="b1")
                    for k in range(9):
                        te.matmul(ps, fmT[:, k, :], wst[:, k, :],
                                  start=(k == 0), stop=(k == 8))
                    dst = c1 if which == "c" else s1r
                    se.activation(out=dst[:, n * 512:(n + 1) * 512], in_=ps,
                                  func=AF.Tanh)
            ve.tensor_mul(s1r, s1r, ig1)
            ve.tensor_add(s1r, s1r, nb1)
            tp = psB.tile([128, 8 * 16], F32, tag="b1")
            transpose_16cols(s1r, 8, tp)
            h1T = fresh_hT("h1T")
            ps_copy(h1T[:, :8, :], tp.rearrange("p (k b) -> p k b", b=16))

            # xW1 = xT.T @ W1emb.T + bias1 -> DRAM scratch f16
            for n in range(8):
                wst = stream.tile([128, 5, 512], F16, tag="wst")
                sync.dma_start(out=wst, in_=D["W1embT"][n])
                for m in range(3):
                    mw = 128 if m < 2 else ROWS - 256
                    ps = psA.tile([128, 512], F32, tag="gates")
                    for k in range(5):
                        te.matmul(ps[:mw], xtT[:, k, m * 128:m * 128 + mw],
                                  wst[:, k, :], start=(k == 0), stop=(k == 4))
                    sst = outp.tile([128, 512], F16, tag="sst")
                    ps_copy(sst[:mw], ps[:mw])
                    sync.dma_start(
                        out=xw_dram[m, :mw, n * 512:(n + 1) * 512],
                        in_=sst[:mw])

        if dbg:
            sync.dma_start(out=dbg["vp"], in_=vp)
            sync.dma_start(out=dbg["uvT"], in_=uvT)
            sync.dma_start(out=dbg["fm"], in_=fm)
            sync.dma_start(out=dbg["c1"], in_=c1)
            sync.dma_start(out=dbg["h1T"], in_=h1T)

        # resident weights (after phase-0 pool frees its space)
        whh2 = resid.tile([128, 8, G], F16)
        sync.dma_start(out=whh2, in_=D["Whh2T"])
        attw = resid.tile([128, 9, ATT], F16)
        sync.dma_start(out=attw, in_=D["attWT"])
        projw = resid.tile([128, 9, VDIM], F16)
        sync.dma_start(out=projw, in_=D["projWT"])

        h2T = hT.tile([128, 9, BL], F16, tag="h2T")
        sync.dma_start(out=h2T, in_=D["h2Tinit"])

        # =============== decode loop ===============
        for t in range(TS):
            xw = xwp.tile([BL, G], F16, tag="xw")
            sync.dma_start(out=xw,
                           in_=xw_dram[t // 8, (t % 8) * 16:(t % 8) * 16 + 16, :])

            if dbg and t == 0:
                sync.dma_start(out=dbg["xw"], in_=xw)

            # --- attention ---
            psq = psB.tile([BL, ATT], F32, tag="b1")
            for k in range(9):
                te.matmul(psq, h2T[:, k, :], attw[:, k, :],
                          start=(k == 0), stop=(k == 8))
            qsb = trans.tile([BL, ATT], F32, tag="cellg")
            ps_copy(qsb, psq)
            if dbg and t == 0:
                sync.dma_start(out=dbg["q0"], in_=qsb)
            psqt = psB.tile([128, 4 * 16], F32, tag="b1")
            transpose_16cols(qsb, 4, psqt)
            qtf = trans.tile([128, 4 * 16], F16, tag="qtf")
            ps_copy(qtf, psqt)

            ssb = trans.tile([128, 2, BN], F16, tag="ssb")
            pse0 = psB.tile([1, 512], F32, tag="b1")
            pse1 = psB.tile([1, 512], F32, tag="b1")
            pse = (pse0, pse1)
            for j in range(4):
                jj = j % 2
                qb = qtf[:, j * 16:(j + 1) * 16]
                qbc = bass.AP(tensor=qb.tensor, offset=qb.offset,
                              ap=[qb.ap[0], qb.ap[1], [0, N]])
                ve.tensor_tensor(
                    out=ssb[:, jj, :].rearrange("p (b n) -> p b n", n=N),
                    in0=uvT[:, j, :].rearrange("p (b n) -> p b n", n=N),
                    in1=qbc, op=ALU.add)
                se.activation(out=ssb[:, jj, :], in_=ssb[:, jj, :], func=AF.Tanh)
                for hh in range(2):
                    te.matmul(pse[hh][:, :392],
                              attv_t[:, j:j + 1],
                              ssb[:, jj, hh * 392:(hh + 1) * 392],
                              start=(j == 0), stop=(j == 3))

            # softmax over n within each batch (half hh holds batches 8hh..8hh+7)
            mx = trans.tile([1, BL], F32, tag="mx")
            ex = trans.tile([1, BN], F32, tag="ex")
            sm = trans.tile([1, BL], F32, tag="sm")
            for hh in range(2):
                ve.reduce_max(
                    out=mx[:, hh * 8:(hh + 1) * 8],
                    in_=pse[hh][:, :392].rearrange("p (b n) -> p b n", n=N),
                    axis=mybir.AxisListType.X)
                mxh = mx[:, hh * 8:(hh + 1) * 8]
                mxb = bass.AP(tensor=mxh.tensor, offset=mxh.offset,
                              ap=[mxh.ap[0], mxh.ap[1], [0, N]])
                ve.tensor_tensor(
                    out=ex[:, hh * 392:(hh + 1) * 392].rearrange(
                        "p (b n) -> p b n", n=N),
                    in0=pse[hh][:, :392].rearrange("p (b n) -> p b n", n=N),
                    in1=mxb, op=ALU.subtract)
            se.activation(out=ex, in_=ex, func=AF.Exp)
            ve.reduce_sum(out=sm, in_=ex.rearrange("p (b n) -> p b n", n=N),
                          axis=mybir.AxisListType.X)
            ve.reciprocal(out=sm, in_=sm)
            smb = bass.AP(tensor=sm.tensor, offset=sm.offset,
                          ap=[sm.ap[0], [1, BL], [0, N]])
            ve.tensor_tensor(out=ex.rearrange("p (b n) -> p b n", n=N),
                             in0=ex.rearrange("p (b n) -> p b n", n=N),
                             in1=smb, op=ALU.mult)

            if dbg and t == 0:
                sync.dma_start(out=dbg["ex0"], in_=ex)

            # scatter a into block-diagonal A (f16)
            psat = psB.tile([N, BL], F32, tag="b1")
            for b in range(BL):
                te.transpose(psat[:, b:b + 1], ex[:, b * N:(b + 1) * N],
                             ident[:1, :1])
            atf = trans.tile([N, BL], F16, tag="atf")
            ve.tensor_copy(out=atf, in_=psat)
            # amat[n + 49*(b%2), b//2, b] = atf[n, b]; b = 2j + pb
            for pb in range(2):
                dst = amat[pb * N:(pb + 1) * N]          # [49, 8, 16]
                dstv = bass.AP(tensor=dst.tensor, offset=dst.offset + pb,
                               ap=[dst.ap[0], [18, 8]])
                srcv = bass.AP(tensor=atf.tensor, offset=atf.offset + pb,
                               ap=[atf.ap[0], [2, 8]])
                sync.dma_start(out=dstv, in_=srcv)

            # ctx = A.T @ Vp
            csb = trans.tile([BL, H], F32, tag="cellh")
            for n in range(2):
                psc = psB.tile([BL, 512], F32, tag="b1")
                for i in range(NBNT):
                    te.matmul(psc, amat[:, i, :],
                              vp[:, i, n * 512:(n + 1) * 512],
                              start=(i == 0), stop=(i == NBNT - 1))
                ps_copy(csb[:, n * 512:(n + 1) * 512], psc)
            if dbg and t == 0:
                sync.dma_start(out=dbg["ctx0"], in_=csb)
            psct = psB.tile([128, 8 * 16], F32, tag="b1")
            transpose_16cols(csb, 8, psct)
            ctxT = trans.tile([128, 8, BL], F16, tag="ctxT")
            ps_copy(ctxT, psct.rearrange("p (k b) -> p k b", b=16))

            # --- layer 1 gates (n-outer; i,f,g,o = banks 0-1,2-3,4-5,6-7) ---
            gb1 = trans.tile([BL, G], F32, tag="cellg")
            for n in range(8):
                psn = psA.tile([BL, 512], F32, tag="gates")
                wst = stream.tile([128, 8, 512], F16, tag="wst")
                sync.dma_start(out=wst, in_=D["W1ctxT"][n])
                for k in range(8):
                    te.matmul(psn, ctxT[:, k, :], wst[:, k, :],
                              start=(k == 0), stop=False)
                wst2 = stream.tile([128, 8, 512], F16, tag="wst")
                sync.dma_start(out=wst2, in_=D["Whh1T"][n])
                for k in range(8):
                    te.matmul(psn, h1T[:, k, :], wst2[:, k, :],
                              start=False, stop=False)
                te.matmul(psn, i16, xw[:, n * 512:(n + 1) * 512],
                          start=False, stop=True)
                _gate_act(nc, gb1, psn, n)

            if dbg and t == 0:
                sync.dma_start(out=dbg["g10"], in_=gb1)
            s1 = _cell_fin(nc, trans, gb1, c1, eps)
            tp1 = psB.tile([128, 8 * 16], F32, tag="b1")
            transpose_16cols(s1, 8, tp1)
            h1T = fresh_hT("h1T")
            ps_copy(h1T[:, :8, :], tp1.rearrange("p (k b) -> p k b", b=16))

            if dbg and t == 0:
                sync.dma_start(out=dbg["s10"], in_=h1T)

            # --- layer 2 gates ---
            gb2 = trans.tile([BL, G], F32, tag="cellg")
            for n in range(8):
                psn = psA.tile([BL, 512], F32, tag="gates")
                wst = stream.tile([128, 9, 512], F16, tag="wst")
                sync.dma_start(out=wst, in_=D["W2ihT"][n])
                for k in range(9):
                    te.matmul(psn, h1T[:, k, :], wst[:, k, :],
                              start=(k == 0), stop=False)
                for k in range(8):
                    te.matmul(psn, h2T[:, k, :],
                              whh2[:, k, n * 512:(n + 1) * 512],
                              start=False, stop=(k == 7))
                _gate_act(nc, gb2, psn, n)

            s2 = _cell_fin(nc, trans, gb2, c2, eps)
            tp2 = psB.tile([128, 8 * 16], F32, tag="b1")
            transpose_16cols(s2, 8, tp2)
            h2T = fresh_hT("h2T")
            ps_copy(h2T[:, :8, :], tp2.rearrange("p (k b) -> p k b", b=16))

            if dbg and t == 0:
                sync.dma_start(out=dbg["s20"], in_=h2T)

            # --- P = s2 @ projW.T (store transposed for logits) ---
            psp = psB.tile([BL, VDIM], F32, tag="b1")
            for k in range(9):
                te.matmul(psp, h2T[:, k, :], projw[:, k, :],
                          start=(k == 0), stop=(k == 8))
            psb_ = trans.tile([BL, VDIM], F32, tag="cellg")
            ps_copy(psb_, psp)
            pst = psB.tile([128, 4 * 16], F32, tag="b1")
            transpose_16cols(psb_, 4, pst)
            ps_copy(ptT[:, :, t * 16:(t + 1) * 16],
                    pst.rearrange("p (k b) -> p k b", b=16))

        # =============== logits ===============
        for c in range(20):
            et = stream.tile([128, 4, 512], F16, tag="wst")
            sync.dma_start(out=et, in_=D["ET"][c])
            for m in range(3):
                mw = 128 if m < 2 else ROWS - 256
                pso = psB.tile([128, 512], F32, tag="b1")
                for k in range(4):
                    te.matmul(pso[:mw], ptT[:, k, m * 128:m * 128 + mw],
                              et[:, k, :], start=(k == 0), stop=(k == 3))
                osb = outp.tile([128, 512], F32, tag="osb")
                ps_copy(osb[:mw], pso[:mw])
                sync.dma_start(out=out_ap[m * 128:m * 128 + mw,
                                          c * 512:(c + 1) * 512],
                               in_=osb[:mw])


def _gate_act(nc, gb, psn, n):
    """ACT bank n of the gate psum into packed gate tile gb [16, 4096].

    Free-dim quarters: 0=i, 1024=f, 2048=g, 3072=o.
    """
    func = AF.Tanh if n in (4, 5) else AF.Sigmoid
    nc.scalar.activation(out=gb[:, n * 512:(n + 1) * 512], in_=psn, func=func)


def _cell_fin(nc, trans, gb, c_state, eps):
    """c/h update + standardized LN from packed gate tile. Returns norm h."""
    ve, se = nc.vector, nc.scalar
    ig, fg = gb[:, 0:H], gb[:, H:2 * H]
    gg, og = gb[:, 2 * H:3 * H], gb[:, 3 * H:4 * H]
    ve.tensor_mul(c_state, fg, c_state)
    ve.tensor_mul(gg, ig, gg)
    ve.tensor_add(c_state, c_state, gg)
    hb = trans.tile([BL, 2, H], F32, tag="cellh")
    hr, tc_ = hb[:, 0, :], hb[:, 1, :]
    se.activation(out=tc_, in_=c_state, func=AF.Tanh)
    ve.tensor_mul(hr, og, tc_)
    # layer norm (standardize only; gamma/beta folded into consumers)
    st = trans.tile([BL, 2, 6], F32, tag="cellst")
    ve.bn_stats(out=st[:, 0, :], in_=hr[:, 0:512])
    ve.bn_stats(out=st[:, 1, :], in_=hr[:, 512:1024])
    mv = trans.tile([BL, 2], F32, tag="cellmv")
    ve.bn_aggr(out=mv, in_=st)
    sd = trans.tile([BL, 1], F32, tag="cellsd")
    se.activation(out=sd, in_=mv[:, 1:2], func=AF.Sqrt, bias=eps)
    ve.reciprocal(out=sd, in_=sd)
    ve.tensor_scalar(out=hr, in0=hr, scalar1=mv[:, 0:1], scalar2=sd,
                     op0=ALU.subtract, op1=ALU.mult)
    return hr


# ---------------------------------------------------------------------------
# host side
# ---------------------------------------------------------------------------

def _prep_inputs(V, y, embed_W, Vp_W, Vp_b, attW, attU, attv,
                 l1_Wih, l1_Whh, l1_bih, l1_bhh,
                 l2_Wih, l2_Whh, l2_bih, l2_bhh,
                 n1_g, n1_b, n2_g, n2_b,
                 ih_W, ih_b, ic_W, ic_b, proj_W):
    f8 = np.float64
    g1, b1v = np.asarray(n1_g, f8), np.asarray(n1_b, f8)
    g2, b2v = np.asarray(n2_g, f8), np.asarray(n2_b, f8)
    W1 = np.asarray(l1_Wih, f8)
    Whh1 = np.asarray(l1_Whh, f8) * g1[None, :]
    W2ih = np.asarray(l2_Wih, f8) * g1[None, :]
    Whh2 = np.asarray(l2_Whh, f8) * g2[None, :]
    bias1 = (np.asarray(l1_bih, f8) + np.asarray(l1_bhh, f8)
             + b1v @ np.asarray(l1_Whh, f8).T)
    bias2 = (np.asarray(l2_bih, f8) + np.asarray(l2_bhh, f8)
             + b1v @ np.asarray(l2_Wih, f8).T + b2v @ np.asarray(l2_Whh, f8).T)
    attW_e = np.asarray(attW, f8) * g2[None, :]
    bq = b2v @ np.asarray(attW, f8).T
    projW_e = np.asarray(proj_W, f8) * g2[None, :]
    bp = b2v @ np.asarray(proj_W, f8).T
    E = np.asarray(embed_W, f8)
    M1 = np.asarray(Vp_W, f8).T @ np.asarray(attU, f8).T    # [512, 512]
    bUv = np.asarray(Vp_b, f8) @ np.asarray(attU, f8).T     # [512]

    shared = {
        "VpWT": _f16(_ktile(_aug(np.asarray(Vp_W, f8).T, np.asarray(Vp_b, f8)))),
        "M1": _f16(_ktile(M1)),
        "bU": _f32(bUv.reshape(4, 128).T),
        "ihWT": _f16(_nmaj(_aug(np.asarray(ih_W, f8).T, np.asarray(ih_b, f8)))),
        "icWT": _f16(_nmaj(_aug(np.asarray(ic_W, f8).T, np.asarray(ic_b, f8)))),
        "invG1": _f32(np.broadcast_to((1.0 / g1)[None, :], (BL, H))),
        "nB1G1": _f32(np.broadcast_to((-b1v / g1)[None, :], (BL, H))),
        "W1embT": _f16(_nmaj(_aug(W1[:, :EMB].T, bias1))),
        "W1ctxT": _f16(_nmaj(W1[:, EMB:].T)),
        "Whh1T": _f16(_nmaj(Whh1.T)),
        "W2ihT": _f16(_nmaj(_aug(W2ih.T, bias2))),
        "Whh2T": _f16(_ktile(Whh2.T)),
        "attWT": _f16(_ktile(_aug(attW_e.T, bq))),
        "attv": _f16(np.asarray(attv, f8)[0].reshape(4, 128).T),
        "projWT": _f16(_ktile(_aug(projW_e.T, bp))),
    }
    ETp = np.zeros((EMB, VP), f8)
    ETp[:, :VOCAB] = E.T
    shared["ET"] = _f16(ETp.reshape(4, 128, 20, 512).transpose(2, 1, 0, 3))
    am = np.zeros((BN, BL), f8)
    for b in range(BL):
        am[b * N:(b + 1) * N, b] = 1.0 / N
    shared["Amean"] = _f16(am.reshape(NBNT, BNT, BL).transpose(1, 0, 2))
    s2i = np.broadcast_to((-b2v / g2)[:, None], (H, BL))     # [1024, 16]
    h2i = np.zeros((128, 9, BL), f8)
    h2i[:, :8, :] = s2i.reshape(8, 128, BL).transpose(1, 0, 2)
    h2i[0, 8, :] = 1.0
    shared["h2Tinit"] = _f16(h2i)

    emb_all = np.asarray(embed_W, np.float32)[np.asarray(y).astype(np.int64)]
    in_maps = []
    for c in range(NCORES):
        Vl = np.asarray(V, f8)[c * BL:(c + 1) * BL]          # [16, 49, 512]
        VT = Vl.reshape(BN, VDIM).T                          # [512, 784]
        xl = emb_all[c * BL:(c + 1) * BL, :TS]               # [16, 19, 512]
        xT = xl.transpose(2, 1, 0).reshape(EMB, ROWS).astype(f8)
        m = dict(shared)
        m["VT"] = _f16(_ktile(_aug(VT, 1.0)))
        m["xT"] = _f16(_ktile(_aug(xT, 1.0)))
        in_maps.append(m)
    return in_maps


class _Exec:
    def __init__(self, nc):
        install_neuronx_cc_hook()
        self.nc = nc
        pname = nc.partition_id_tensor.name if nc.partition_id_tensor else None
        in_names, out_names, out_avals = [], [], []
        for alloc in nc.m.functions[0].allocations:
            if not isinstance(alloc, mybir.MemoryLocationSet):
                continue
            name = alloc.memorylocations[0].name
            if alloc.kind == "ExternalInput":
                if name != pname:
                    in_names.append(name)
            elif alloc.kind == "ExternalOutput":
                out_names.append(name)
                out_avals.append(jax.core.ShapedArray(
                    tuple(alloc.tensor_shape), mybir.dt.np(alloc.dtype)))
        self.in_names, self.out_names, self.out_avals = in_names, out_names, out_avals
        all_in = list(in_names) + list(out_names) + ([pname] if pname else [])

        def _body(*args):
            operands = list(args)
            if pname is not None:
                operands.append(partition_id_tensor())
            return tuple(_bass_exec_p.bind(
                *operands, out_avals=tuple(out_avals), in_names=tuple(all_in),
                out_names=tuple(out_names), lowering_input_output_aliases=(),
                sim_require_finite=True, sim_require_nnan=True, nc=nc))

        devices = jax.devices()[:NCORES]
        self.mesh = Mesh(np.asarray(devices), ("core",))
        nio = len(in_names) + len(out_names)
        self.fn = jax.jit(shard_map(_body, mesh=self.mesh,
                                    in_specs=(PartitionSpec("core"),) * nio,
                                    out_specs=(PartitionSpec("core"),) * len(out_names),
                                    check_rep=False), keep_unused=True)

    def put(self, in_maps):
        sh = NamedSharding(self.mesh, PartitionSpec("core"))
        args = []
        for nm in self.in_names:
            args.append(jax.device_put(
                np.concatenate([np.asarray(in_maps[c][nm]) for c in range(NCORES)],
                               axis=0), sh))
        for av in self.out_avals:
            args.append(jax.device_put(
                np.zeros((NCORES * av.shape[0], *av.shape[1:]), av.dtype), sh))
        return args

    def exec_only(self, args):
        outs = self.fn(*args)
        jax.block_until_ready(outs)
        return outs

    def run(self, in_maps):
        return self.exec_only(self.put(in_maps))

    def split(self, outs):
        res = []
        for c in range(NCORES):
            d = {}
            for i, nm in enumerate(self.out_names):
                av = self.out_avals[i]
                d[nm] = np.asarray(outs[i]).reshape(NCORES, *av.shape)[c]
            res.append(d)
        return res


def _get_exec():
    if "exec" not in _cache:
        nc = build_program()
        _cache["exec"] = _Exec(nc)
    return _cache["exec"]


def kernel(**inputs):
    ex = _get_exec()
    in_maps = _prep_inputs(**inputs)
    outs = ex.run(in_maps)
    per_core = ex.split(outs)
    full = np.empty((B, TS, VOCAB), np.float32)
    for c in range(NCORES):
        o = per_core[c]["out"][:, :VOCAB]           # [304, 10000]
        full[c * BL:(c + 1) * BL] = o.reshape(TS, BL, VOCAB).transpose(1, 0, 2)
    return full
